# revision 1
# baseline (speedup 1.0000x reference)
"""Trainium2 Bass kernel for CausalCrossAttention (B=8, T=769, C=1024, H=16).

Sharding: data-parallel over batch B=8 across the 8 NeuronCores (one batch
element per core, SPMD — identical program, different input slices).

Per-core dataflow (all matmuls in fp32r on the PE at 1 cycle/row):
  1. Q/K projections in [c_out, t] layout:
     matmul(out=[co_tile, t], lhsT=W^T[ci, co_tile], rhs=x^T[ci, t]).
     Host pre-transposes W and x so the contraction dim (ci) is the SBUF
     partition dim for both operands (no on-chip transposes anywhere).
     Rotary is applied in [dim, t] layout via a host permutation of W's output
     dims (even/odd pair split) + partition-block-swap DMAs + 3 DVE ops.
  2. V projection in [t, c_out] layout (lhsT=x^T tile, rhs=W^T), written into a
     ones-augmented V buffer (col 64 of each head's 65-wide block = 1.0, so the
     softmax denominator falls out of the PV matmul for free).
  3. Attention per head in S^T layout: S^T[tkv, tq] = k^T.T @ qz where qz is
     q^T with the sibling head's partition rows zeroed -> K=128 matmuls that
     keep the PE activity monitor warm (K=64 ran at the 1.2 GHz cold clock).
     exp on ACT (1/sqrt(hd) folded into the activation scale), prefix-causal
     mask (col < 256 + row) as a triangular multiply on boundary tiles
     (GpSimd), PV with M=128 (V_aug free-dim padded; garbage out rows unread).
  4. Softmax division per head-pair: denominator rows bounce through DRAM,
     partition-broadcast back, one in-place reciprocal per pair, two DVE mults.
  5. Output projection from the Y^T[ci, t] layout that PV naturally produces.

fp32r ISA restrictions honored throughout: moving-operand free count and PSUM
dst offsets even (T streams padded to 770), producers of matmul operands write
float32r-typed tiles.
"""

import os

import numpy as np

B, T, C = 8, 769, 1024
H, HD, L = 16, 64, 32
COND = 256
NCI = 8  # 1024 / 128 contraction tiles
NCO = 8
NTT = 7  # t tiles: 6 full + 1 single row
TP = 770  # streamed T padded to even (fp32r matmul: moving N must be even)
R0 = (0, 512)
R1 = (512, 770)
VW = H * (HD + 1) + 63  # V_aug free width, padded so M=128 slices stay in-bounds

_CACHE = {}


def _build_program():
    import concourse.mybir as mybir
    import concourse.tile as tile
    from concourse import bacc

    f32 = mybir.dt.float32
    f32r = mybir.dt.float32r
    Exp = mybir.ActivationFunctionType.Exp
    Ident = mybir.ActivationFunctionType.Identity
    Copy = mybir.ActivationFunctionType.Copy

    nc = bacc.Bacc("TRN2", target_bir_lowering=False)

    xqT_d = nc.dram_tensor("xqT", [C, TP], f32r, kind="ExternalInput")
    xkvT_d = nc.dram_tensor("xkvT", [C, TP], f32r, kind="ExternalInput")
    wq_d = nc.dram_tensor("wqT", [C, C], f32r, kind="ExternalInput")
    wk_d = nc.dram_tensor("wkT", [C, C], f32r, kind="ExternalInput")
    wv_d = nc.dram_tensor("wvT", [C, C], f32r, kind="ExternalInput")
    wp_d = nc.dram_tensor("wpT", [C, C], f32r, kind="ExternalInput")
    bq_d = nc.dram_tensor("bq2", [128, NCO], f32, kind="ExternalInput")
    bk_d = nc.dram_tensor("bk2", [128, NCO], f32, kind="ExternalInput")
    bv_d = nc.dram_tensor("bv1", [1, C], f32, kind="ExternalInput")
    bp_d = nc.dram_tensor("bp1", [1, C], f32, kind="ExternalInput")
    cos_d = nc.dram_tensor("cosP", [128, TP], f32, kind="ExternalInput")
    sin_d = nc.dram_tensor("sinP", [128, TP], f32, kind="ExternalInput")
    m0_d = nc.dram_tensor("m0", [128, 128], f32, kind="ExternalInput")
    out_d = nc.dram_tensor("out", [T, C], f32, kind="ExternalOutput")

    # Per-(kv-tile) q ranges in the 0:512 block + mask offset.
    # nk covers kv cols [128*nk, 128*nk+128); allowed iff kv_col < 256 + q_col,
    # i.e. p < f + 256 - 128*nk with p the in-tile kv index, f the abs q col.
    R0SUB = {0: (0, 512, None), 1: (0, 512, None), 2: (0, 512, 0),
             3: (128, 512, 128), 4: (256, 512, 256), 5: (384, 512, 384)}

    with tile.TileContext(nc) as tc:
        with (
            tc.tile_pool(name="consts", bufs=1) as consts,
            tc.tile_pool(name="wpool", bufs=1) as wpool,
            tc.tile_pool(name="qkpool", bufs=1) as qkpool,
            tc.tile_pool(name="vpool", bufs=1) as vpool,
        ):
            cos_sb = consts.tile([128, TP], f32, tag="cos")
            sin_sb = consts.tile([128, TP], f32, tag="sin")
            m0_sb = consts.tile([128, 128], f32, tag="m0")
            bq_sb = consts.tile([128, NCO], f32, tag="bq")
            bk_sb = consts.tile([128, NCO], f32, tag="bk")
            ones16 = consts.tile([128, 16], f32, tag="ones16")
            nc.vector.memset(ones16, 1.0)
            zcol = consts.tile([128, TP], f32, tag="zcol")
            nc.vector.memset(zcol, 0.0)
            nc.scalar.dma_start(out=cos_sb, in_=cos_d[:, :])
            nc.scalar.dma_start(out=sin_sb, in_=sin_d[:, :])
            nc.scalar.dma_start(out=m0_sb, in_=m0_d[:, :])
            nc.scalar.dma_start(out=bq_sb, in_=bq_d[:, :])
            nc.scalar.dma_start(out=bk_sb, in_=bk_d[:, :])

            qT = qkpool.tile([128, NCI, TP], f32r, tag="qT")
            kT = qkpool.tile([128, NCI, TP], f32r, tag="kT")
            vaug = vpool.tile([128, NTT, VW], f32r, tag="vaug")
            yT = None  # allocated in the phase-2/3 scope

            def load_w(wdram, pfx):
                # half-width tiles: each slot is released after its last read
                # so the next projection's weight DMAs overlap this one.
                ws = []
                for ci in range(NCI):
                    row = []
                    for hf in (0, 1):
                        wt = wpool.tile([128, 512], f32r, tag=f"w{ci}h{hf}",
                                        name=f"{pfx}{ci}h{hf}")
                        nc.sync.dma_start(
                            out=wt,
                            in_=wdram[ci * 128:(ci + 1) * 128,
                                      hf * 512:(hf + 1) * 512])
                        row.append(wt)
                    ws.append(row)
                return ws

            def proj_qk(w, x, b_sb, outT, psA, shpool):
                """outT[:, co, :] = W @ x^T + b, then partial rotary.

                ci-outer so each w[ci] slot is released early and the next
                projection's weight DMAs overlap this projection's compute.
                """
                for cog in (range(0, 4), range(4, 8)):
                    pss = {}
                    for co in cog:
                        pss[co] = psA.tile([128, 1024], f32, tag="ps",
                                           name=f"psqk{co}")
                    for ci in range(NCI):
                        for co in cog:
                            lhs = w[ci][co // 4][
                                :, (co % 4) * 128:(co % 4 + 1) * 128]
                            for (lo, hi) in (R0, R1):
                                nc.tensor.matmul(
                                    pss[co][:, lo:hi], lhs, x[:, ci, lo:hi],
                                    start=(ci == 0), stop=(ci == NCI - 1))
                    for co in cog:
                        proj_qk_tail(pss[co], co, b_sb, outT, shpool)

            def proj_qk_tail(ps, co, b_sb, outT, shpool):
                # bias add + PSUM->SBUF on ACT
                nc.scalar.activation(
                    out=outT[:, co, :], in_=ps[:, 0:TP], func=Ident,
                    bias=b_sb[:, co:co + 1], scale=1.0)
                # rotary: swap 16-row blocks of the (host-permuted) rotary
                # dims, then q = q*cos + swapped*sin.  cos rows outside the
                # rotary dims are 1.0, sin rows are 0.0, so full-height DVE
                # ops are safe (and cost the same as 32-row ones).
                sh = shpool.tile([128, TP], f32r, tag="sh", name="sh")
                nc.sync.dma_start(
                    out=sh[32:64, :], in_=outT[32:64, co, :])
                for s in (0, 64):
                    nc.sync.dma_start(
                        out=sh[s:s + 16, :], in_=outT[s + 16:s + 32, co, :])
                    nc.sync.dma_start(
                        out=sh[s + 16:s + 32, :], in_=outT[s:s + 16, co, :])
                nc.vector.tensor_mul(
                    sh[0:96, :], sh[0:96, :], sin_sb[0:96, :])
                nc.vector.tensor_mul(
                    outT[:, co, :], outT[:, co, :], cos_sb)
                nc.vector.tensor_add(
                    outT[0:96, co, :], outT[0:96, co, :], sh[0:96, :])

            def proj_v(w, x, psA, bv_sb):
                for g in (range(0, 4), range(4, 7)):
                    pss = {}
                    for tt in g:
                        pss[tt] = psA.tile([128, 1024], f32, tag="ps",
                                           name=f"psv{tt}")
                    for ci in range(NCI):
                        for tt in g:
                            tsz = 128 if tt < 6 else 1
                            lhs = x[:, ci, tt * 128:tt * 128 + tsz]
                            for hf in (0, 1):
                                nc.tensor.matmul(
                                    pss[tt][:tsz, hf * 512:hf * 512 + 512],
                                    lhs, w[ci][hf],
                                    start=(ci == 0), stop=(ci == NCI - 1))
                    for tt in g:
                        tsz = 128 if tt < 6 else 1
                        va = vaug[:tsz, tt, 0:H * (HD + 1)].rearrange(
                            "p (h e) -> p h e", e=HD + 1)
                        nc.vector.tensor_add(
                            va[:, :, 0:HD],
                            pss[tt][:tsz, :].rearrange("p (h d) -> p h d", h=H),
                            bv_sb[:tsz, :].rearrange("p (h d) -> p h d", h=H))
                        nc.vector.tensor_copy(
                            va[:, :, HD:HD + 1], ones16[:tsz, :].unsqueeze(2))
                        # pad tail so M=128 lhsT slices stay initialized
                        nc.vector.tensor_copy(
                            vaug[:tsz, tt, H * (HD + 1):VW],
                            zcol[:tsz, 0:VW - H * (HD + 1)])

            def attn(h, qz, pt_pool, psS, psO, dnd, stg_pool):
                j, s = h // 2, 64 * (h % 2)
                pts = {}
                for nk in range(6):
                    pt = pt_pool.tile([128, TP], f32r, tag=f"pt{nk}",
                                      name=f"pt{nk}")
                    pts[nk] = pt
                    qlo, qhi, moff = R0SUB[nk]
                    ps = psS.tile([128, 1024], f32, tag="st", name=f"st{nk}")
                    nc.tensor.matmul(
                        ps[:, qlo:qhi], kT[:, j, nk * 128:(nk + 1) * 128],
                        qz[:, qlo:qhi], start=True, stop=True)
                    nc.tensor.matmul(
                        ps[:, 512:770], kT[:, j, nk * 128:(nk + 1) * 128],
                        qz[:, 512:770], start=True, stop=True)
                    # one exp over both contiguous q ranges
                    nc.scalar.activation(
                        out=pt[:, qlo:770], in_=ps[:, qlo:770],
                        func=Exp, scale=0.125)
                    if moff is not None:
                        nc.gpsimd.tensor_mul(
                            pt[:, moff:moff + 128], pt[:, moff:moff + 128],
                            m0_sb)
                # kv col 768 (single kv row): q col 512 is masked; stream the
                # full even range and zero that one probability instead.
                pt6 = pt_pool.tile([128, TP], f32r, tag="pt6", bufs=1)
                ps = psS.tile([128, 1024], f32, tag="st", name="st6")
                nc.tensor.matmul(
                    ps[0:1, 0:258], kT[:, j, 768:769], qz[:, 512:770],
                    start=True, stop=True)
                nc.scalar.activation(
                    out=pt6[0:1, 513:770], in_=ps[0:1, 1:258],
                    func=Exp, scale=0.125)
                nc.vector.tensor_copy(pt6[0:1, 512:513], zcol[0:1, 0:1])

                # PV (+denominator via the ones column) into one merged psum.
                # lhsT is M=128 wide (only out rows 0:65 are meaningful).
                vs = slice(h * (HD + 1), h * (HD + 1) + 128)
                o = psO.tile([128, 1024], f32, tag="ov", name=f"ov{h % 2}")
                for nk in range(6):
                    qlo, qhi, _ = R0SUB[nk]
                    nc.tensor.matmul(
                        o[:, qlo:qhi], vaug[:, nk, vs], pts[nk][:, qlo:qhi],
                        start=(nk == 0), stop=False)
                    nc.tensor.matmul(
                        o[:, 512:770], vaug[:, nk, vs], pts[nk][:, 512:770],
                        start=(nk == 0), stop=False)
                nc.tensor.matmul(
                    o[:, 512:770], vaug[0:1, 6, vs], pt6[0:1, 512:770],
                    start=False, stop=True)

                # denominator row to DRAM (via base-0 staging)
                stg = stg_pool.tile([1, TP], f32, tag="dstage", bufs=4,
                                    name=f"stg{h}")
                nc.scalar.activation(
                    out=stg[0:1, 0:770], in_=o[HD:HD + 1, 0:770], func=Copy)
                nc.sync.dma_start(out=dnd[h:h + 1, :], in_=stg[0:1, :])
                return o

            def proj_out(w, psA, opool, bp_sb):
                for g in (range(0, 4), range(4, 7)):
                    pss = {}
                    for tt in g:
                        pss[tt] = psA.tile([128, 1024], f32, tag="pso",
                                           name=f"pso{tt}")
                    for ci in range(NCI):
                        for tt in g:
                            tsz = 128 if tt < 6 else 1
                            lhs = yT[:, ci, tt * 128:tt * 128 + tsz]
                            for hf in (0, 1):
                                nc.tensor.matmul(
                                    pss[tt][:tsz, hf * 512:hf * 512 + 512],
                                    lhs, w[ci][hf],
                                    start=(ci == 0), stop=(ci == NCI - 1))
                    for tt in g:
                        tsz = 128 if tt < 6 else 1
                        ot = opool.tile([128, 1024], f32, tag="ot", name="ot")
                        nc.vector.tensor_add(
                            ot[:tsz, :], pss[tt][:tsz, :], bp_sb[:tsz, :])
                        nc.sync.dma_start(
                            out=out_d[tt * 128:tt * 128 + tsz, :],
                            in_=ot[:tsz, :])

            # ---- phase 1: projections ----
            with (
                tc.tile_pool(name="psA", bufs=4, space="PSUM") as psA,
                tc.tile_pool(name="xq", bufs=1) as xqp,
                tc.tile_pool(name="xkv", bufs=1) as xkp,
                tc.tile_pool(name="shpool", bufs=2) as shpool,
            ):
                bv_sb = xqp.tile([128, C], f32, tag="bv")
                nc.gpsimd.dma_start(
                    out=bv_sb, in_=bv_d[0:1, :].broadcast_to((128, C)))
                xq = xqp.tile([128, NCI, TP], f32r, tag="xq")
                xkv = xkp.tile([128, NCI, TP], f32r, tag="xkv")
                # per-ci-tile DMAs so the first matmuls start early;
                # xkv goes on the SWDGE queue to run parallel with wq/xq.
                for ci in range(NCI):
                    nc.sync.dma_start(
                        out=xq[:, ci, :],
                        in_=xqT_d[ci * 128:(ci + 1) * 128, :])
                    nc.gpsimd.dma_start(
                        out=xkv[:, ci, :],
                        in_=xkvT_d[ci * 128:(ci + 1) * 128, :])
                wq = load_w(wq_d, "wq")
                proj_qk(wq, xq, bq_sb, qT, psA, shpool)
                wk = load_w(wk_d, "wk")
                proj_qk(wk, xkv, bk_sb, kT, psA, shpool)
                wv = load_w(wv_d, "wv")
                proj_v(wv, xkv, psA, bv_sb)

            # ---- phases 2+3 share yT ----
            with tc.tile_pool(name="ypool", bufs=1) as ypool:
              yT = ypool.tile([128, NCI, TP], f32r, tag="yT")
              # ---- phase 2: attention ----
              with (
                tc.tile_pool(name="ptp", bufs=1) as pt_pool,
                tc.tile_pool(name="qzp", bufs=2) as qz_pool,
                tc.tile_pool(name="psS", bufs=2, space="PSUM") as psS,
                tc.tile_pool(name="psO", bufs=2, space="PSUM") as psO,
                tc.tile_pool(name="stgp", bufs=1) as stg_pool,
                tc.tile_pool(name="rdbcp", bufs=2) as rdbc_pool,
                tc.tile_pool(name="rddp", bufs=1, space="DRAM") as dram_pool,
              ):
                wp = load_w(wp_d, "wp")  # prefetch during attention
                dnd = dram_pool.tile([H, TP], f32, tag="dnd")
                for j in range(NCI):
                    # qz: per head, the sibling head's rows zeroed -> K=128
                    # S^T matmuls (keeps the PE activity monitor warm).
                    qza = qz_pool.tile([128, TP], f32r, tag="qza",
                                       name=f"qza{j}")
                    nc.vector.tensor_copy(qza[0:64, :], qT[0:64, j, :])
                    nc.vector.tensor_copy(qza[64:128, :], zcol[64:128, :])
                    qzb = qz_pool.tile([128, TP], f32r, tag="qzb",
                                       name=f"qzb{j}")
                    nc.vector.tensor_copy(qzb[0:64, :], zcol[0:64, :])
                    nc.vector.tensor_copy(qzb[64:128, :], qT[64:128, j, :])
                    oa = attn(2 * j, qza, pt_pool, psS, psO, dnd, stg_pool)
                    ob = attn(2 * j + 1, qzb, pt_pool, psS, psO, dnd, stg_pool)
                    # softmax division for this head pair: broadcast the
                    # denominator rows, one in-place approx reciprocal
                    # (~18 correct bits, 5x faster), two mults from PSUM.
                    rdbc = rdbc_pool.tile([128, TP], f32, tag="rdbc",
                                          name=f"rdbc{j}")
                    nc.gpsimd.dma_start(
                        out=rdbc[0:64, :],
                        in_=dnd[2 * j:2 * j + 1, :].broadcast_to((64, TP)))
                    nc.gpsimd.dma_start(
                        out=rdbc[64:128, :],
                        in_=dnd[2 * j + 1:2 * j + 2, :].broadcast_to((64, TP)))
                    nc.vector.reciprocal_approx_fast(out=rdbc, in_=rdbc)
                    nc.vector.tensor_mul(
                        yT[0:64, j, :], oa[0:HD, 0:770], rdbc[0:64, :])
                    nc.vector.tensor_mul(
                        yT[64:128, j, :], ob[0:HD, 0:770], rdbc[64:128, :])

              # ---- phase 3: output projection ----
              with (
                tc.tile_pool(name="psB", bufs=4, space="PSUM") as psB,
                tc.tile_pool(name="opool", bufs=3) as opool,
              ):
                bp_sb = opool.tile([128, C], f32, tag="bp")
                nc.gpsimd.dma_start(
                    out=bp_sb, in_=bp_d[0:1, :].broadcast_to((128, C)))
                proj_out(wp, psB, opool, bp_sb)

    nc.compile()
    return nc


def _host_prep(x_q, x_kv, rotary_pos_emb, Wq, bq, Wk, bk, Wv, bv, Wp, bp):
    f = np.float32
    x_q = np.asarray(x_q, f)
    x_kv = np.asarray(x_kv, f)
    freqs = np.asarray(rotary_pos_emb, f)

    # Even/odd pair-split permutation of the first 32 dims of each head, so
    # rotate_half becomes a 16-partition block swap on chip.
    perm = np.arange(C)
    for h in range(H):
        b0 = h * HD
        blk = np.empty(HD, np.int64)
        blk[0:16] = b0 + np.arange(0, 32, 2)
        blk[16:32] = b0 + np.arange(1, 32, 2)
        blk[32:64] = b0 + np.arange(32, 64)
        perm[b0:b0 + HD] = blk

    def wT(W, p=None):
        W = np.asarray(W, f)
        if p is not None:
            W = W[p, :]
        return np.ascontiguousarray(W.T)

    cosE = np.cos(freqs[:, 0::2]).T  # [16, T]
    cosO = np.cos(freqs[:, 1::2]).T
    sinE = -np.sin(freqs[:, 0::2]).T
    sinO = np.sin(freqs[:, 1::2]).T
    cosP = np.ones((128, TP), f)
    sinP = np.zeros((128, TP), f)
    for s in (0, 64):
        cosP[s:s + 16, :T] = cosE
        cosP[s + 16:s + 32, :T] = cosO
        sinP[s:s + 16, :T] = sinE
        sinP[s + 16:s + 32, :T] = sinO

    p_idx = np.arange(128)[:, None]
    f_idx = np.arange(128)[None, :]
    m0 = (p_idx < f_idx).astype(f)

    bqp = np.asarray(bq, f)[perm]
    bkp = np.asarray(bk, f)[perm]
    shared = {
        "wqT": wT(Wq, perm),
        "wkT": wT(Wk, perm),
        "wvT": wT(Wv),
        "wpT": wT(Wp),
        "bq2": np.ascontiguousarray(bqp.reshape(NCO, 128).T),
        "bk2": np.ascontiguousarray(bkp.reshape(NCO, 128).T),
        "bv1": np.asarray(bv, f).reshape(1, C).copy(),
        "bp1": np.asarray(bp, f).reshape(1, C).copy(),
        "cosP": np.ascontiguousarray(cosP),
        "sinP": np.ascontiguousarray(sinP),
        "m0": np.ascontiguousarray(m0),
    }

    def padT(xt):
        out = np.zeros((C, TP), f)
        out[:, :T] = xt
        return out

    in_maps = []
    for b in range(B):
        m = dict(shared)
        m["xqT"] = padT(x_q[b].T)
        m["xkvT"] = padT(x_kv[b].T)
        in_maps.append(m)
    return in_maps


def kernel(x_q, x_kv, rotary_pos_emb, Wq, bq, Wk, bk, Wv, bv, Wp, bp):
    from concourse.bass_utils import run_bass_kernel_spmd

    if "nc" not in _CACHE:
        _CACHE["nc"] = _build_program()
    nc = _CACHE["nc"]

    in_maps = _host_prep(x_q, x_kv, rotary_pos_emb,
                         Wq, bq, Wk, bk, Wv, bv, Wp, bp)
    trace = os.environ.get("BTK_TRACE", "0") == "1"
    res = run_bass_kernel_spmd(
        nc, in_maps, core_ids=list(range(B)), trace=trace)
    _CACHE["last_result"] = res
    return np.stack([r["out"] for r in res.results], axis=0)



# revision 17
# speedup vs baseline: 1.6722x; 1.6722x over previous
"""Trainium2 Bass kernel for CausalCrossAttention (B=8, T=769, C=1024, H=16).

Sharding: data-parallel over batch B=8 across the 8 NeuronCores (one batch
element per core, SPMD).

v2 design (vs the 340us baseline):
  * All matmul operands in bf16 (PE speed is the same 1 col/cycle as fp32r,
    but DMA bytes halve and DVE elementwise ops run in 2x packed mode).
    Accumulation stays fp32 in PSUM; rel-err budget measured at ~5e-3 in a
    host simulation vs the 2e-2 gate.
  * ACT (scalar engine) runs ONLY the softmax exp (its ~88us is the global
    pacing constraint); every PSUM eviction moved to DVE copies.
  * Emission is hand-woven so the in-order PE queue never blocks long:
    Q/K co-tiles, V tiles, PV and the output projection are spliced between
    attention score matmuls at sub-microsecond granularity, which also keeps
    the PE HAM activity monitor warm (no >3.4us gaps -> stays at 2.4 GHz).
  * PSUM budget (8 banks): one shared projection/PV pool (2 tiles x 2 banks)
    + two score tiles (2 x 2 banks).
  * V is stored ones-augmented per head pair as [a_dims(64), a_den, b_den,
    b_dims(64)] so both heads' PV outputs land partition-aligned for their
    yT eviction (cross-partition compute ops don't compile), and the
    softmax denominator falls out of the PV matmul. Denominator rows go
    PSUM->DRAM by DMA, come back as a partition-broadcast, one approx
    reciprocal + two in-place muls normalize yT.
  * Output projection in [c_out, t] layout (per-partition bias would be
    free; zero biases skip it entirely) -> contiguous DMA; host transposes.
"""

import os

import numpy as np

B, T, C = 8, 769, 1024
H, HD, L = 16, 64, 32
COND = 256
NCI = 8
NTT = 7
TP = 770
PW = 2 * HD + 2  # 130: per-pair augmented V width [a(64), a_den, b_den, b(64)]
VW = 8 * PW      # 1040

# Per-(kv-tile) allowed q ranges in the 0:512 block + mask offset.
R0SUB = {0: (0, 512, None), 1: (0, 512, None), 2: (0, 512, 0),
         3: (128, 512, 128), 4: (256, 512, 256), 5: (384, 512, 384)}

_CACHE = {}


def _build_program(use_bias):
    import concourse.mybir as mybir
    import concourse.tile as tile
    from concourse import bacc

    f32 = mybir.dt.float32
    bf16 = mybir.dt.bfloat16
    Exp = mybir.ActivationFunctionType.Exp

    nc = bacc.Bacc("TRN2", target_bir_lowering=False)

    xqT_d = nc.dram_tensor("xqT", [C, TP], bf16, kind="ExternalInput")
    xkvT_d = nc.dram_tensor("xkvT", [C, TP], bf16, kind="ExternalInput")
    wq_d = nc.dram_tensor("wqT", [C, C], bf16, kind="ExternalInput")
    wk_d = nc.dram_tensor("wkT", [C, C], bf16, kind="ExternalInput")
    wv_d = nc.dram_tensor("wvT", [C, C], bf16, kind="ExternalInput")
    wp_d = nc.dram_tensor("wpT", [C, C], bf16, kind="ExternalInput")
    cos_d = nc.dram_tensor("cosP", [128, TP], bf16, kind="ExternalInput")
    sin_d = nc.dram_tensor("sinP", [128, TP], bf16, kind="ExternalInput")
    m0_d = nc.dram_tensor("m0", [128, 128], bf16, kind="ExternalInput")
    if use_bias:
        bq_d = nc.dram_tensor("bq2", [128, NCI], f32, kind="ExternalInput")
        bk_d = nc.dram_tensor("bk2", [128, NCI], f32, kind="ExternalInput")
        bp_d = nc.dram_tensor("bp2", [128, NCI], f32, kind="ExternalInput")
        bv_d = nc.dram_tensor("bv1", [1, C], f32, kind="ExternalInput")
    out_d = nc.dram_tensor("out", [C, TP], f32, kind="ExternalOutput")

    with tile.TileContext(nc) as tc:
        with (
            tc.tile_pool(name="consts", bufs=1) as consts,
            tc.tile_pool(name="wq", bufs=1) as wqp,
            tc.tile_pool(name="wk", bufs=1) as wkp,
            tc.tile_pool(name="wv", bufs=1) as wvp,
            tc.tile_pool(name="xq", bufs=1) as xqp,
            tc.tile_pool(name="xkv", bufs=1) as xkp,
            tc.tile_pool(name="qk", bufs=1) as qkp,
            tc.tile_pool(name="vpool", bufs=1) as vpool,
            tc.tile_pool(name="ypool", bufs=1) as ypool,
            tc.tile_pool(name="shp", bufs=2) as shp,
            tc.tile_pool(name="ptp", bufs=3) as ptp,
            tc.tile_pool(name="rdp", bufs=2) as rdp,
            tc.tile_pool(name="oout", bufs=2) as ooutp,
            tc.tile_pool(name="psP", bufs=2, space="PSUM") as psP,
            tc.tile_pool(name="psS", bufs=1, space="PSUM") as psS,
            tc.tile_pool(name="dram", bufs=1, space="DRAM") as dram_pool,
        ):
            # ---------- constants + inputs ----------
            cos_sb = consts.tile([128, TP], bf16, tag="cos")
            sin_sb = consts.tile([128, TP], bf16, tag="sin")
            m0_sb = consts.tile([128, 128], bf16, tag="m0")
            nc.scalar.dma_start(out=cos_sb, in_=cos_d[:, :])
            nc.scalar.dma_start(out=sin_sb, in_=sin_d[:, :])
            nc.scalar.dma_start(out=m0_sb, in_=m0_d[:, :])
            if use_bias:
                bq_sb = consts.tile([128, NCI], f32, tag="bq")
                bk_sb = consts.tile([128, NCI], f32, tag="bk")
                bp_sb = consts.tile([128, NCI], f32, tag="bp")
                bv_sb = consts.tile([128, C], f32, tag="bv")
                nc.scalar.dma_start(out=bq_sb, in_=bq_d[:, :])
                nc.scalar.dma_start(out=bk_sb, in_=bk_d[:, :])
                nc.scalar.dma_start(out=bp_sb, in_=bp_d[:, :])
                nc.gpsimd.dma_start(
                    out=bv_sb, in_=bv_d[0:1, :].broadcast_to((128, C)))

            xq = xqp.tile([128, NCI, TP], bf16, tag="xq")
            xkv = xkp.tile([128, NCI, TP], bf16, tag="xkv")
            for ci in range(NCI):
                nc.sync.dma_start(
                    out=xq[:, ci, :], in_=xqT_d[ci * 128:(ci + 1) * 128, :])
                nc.scalar.dma_start(
                    out=xkv[:, ci, :], in_=xkvT_d[ci * 128:(ci + 1) * 128, :])

            def load_w(pool, wdram, pfx, queue):
                ws = []
                for ci in range(NCI):
                    row = []
                    for hf in (0, 1):
                        wt = pool.tile([128, 512], bf16, tag=f"{pfx}{ci}h{hf}")
                        queue.dma_start(
                            out=wt,
                            in_=wdram[ci * 128:(ci + 1) * 128,
                                      hf * 512:(hf + 1) * 512])
                        row.append(wt)
                    ws.append(row)
                return ws

            wq = load_w(wqp, wq_d, "wq", nc.sync)
            wk = load_w(wkp, wk_d, "wk", nc.sync)
            wv = load_w(wvp, wv_d, "wv", nc.scalar)
            # wp reuses wq's buffers (loaded late, after Q/K finish with them)
            wp = []

            qT = qkp.tile([128, NCI, TP], bf16, tag="qT")
            kT = qkp.tile([128, NCI, TP], bf16, tag="kT")
            vaug = vpool.tile([128, NTT, VW], bf16, tag="vaug")
            yT = ypool.tile([128, NCI, TP], bf16, tag="yT")
            # persistent qz double-buffers; complementary halves zeroed once
            qza = qkp.tile([128, 2, TP], bf16, tag="qza")
            qzb = qkp.tile([128, 2, TP], bf16, tag="qzb")
            nc.vector.memset(qza[64:128, :, :], 0.0)
            nc.vector.memset(qzb[0:64, :, :], 0.0)
            stg = rdp.tile([128, 2, TP], f32, tag="stg", bufs=1)
            dnd = dram_pool.tile([H, TP], f32, tag="dnd")

            # ---------- chunk emitters (generators yield per PE quantum) ----
            def gen_qk(which, j):
                """Q or K projection for co tile j + rotary tail."""
                w, x, outT = ((wq, xq, qT) if which == "q" else (wk, xkv, kT))
                ps = psP.tile([128, 1024], f32, tag="ps")
                for cig in range(4):  # 4 quanta of 2ci x 2 matmuls
                    for ci in (2 * cig, 2 * cig + 1):
                        lhs = w[ci][j // 4][:, (j % 4) * 128:(j % 4 + 1) * 128]
                        nc.tensor.matmul(ps[:, 0:512], lhs, x[:, ci, 0:512],
                                         start=(ci == 0), stop=(ci == 7))
                        nc.tensor.matmul(ps[:, 512:770], lhs, x[:, ci, 512:770],
                                         start=(ci == 0), stop=(ci == 7))
                    yield
                # eviction + rotary (DVE + small SBUF->SBUF swap DMAs)
                if use_bias:
                    b_sb = bq_sb if which == "q" else bk_sb
                    nc.vector.tensor_scalar_add(
                        outT[:, j, :], ps[:, 0:770], b_sb[:, j:j + 1])
                else:
                    nc.vector.tensor_copy(outT[:, j, :], ps[:, 0:770])
                sh = shp.tile([128, TP], bf16, tag="sh")
                nc.sync.dma_start(out=sh[32:64, :], in_=outT[32:64, j, :])
                for s in (0, 64):
                    nc.sync.dma_start(
                        out=sh[s:s + 16, :], in_=outT[s + 16:s + 32, j, :])
                    nc.sync.dma_start(
                        out=sh[s + 16:s + 32, :], in_=outT[s:s + 16, j, :])
                nc.vector.tensor_mul(sh[0:96, :], sh[0:96, :], sin_sb[0:96, :])
                nc.vector.tensor_mul(outT[:, j, :], outT[:, j, :], cos_sb)
                nc.vector.tensor_add(
                    outT[0:96, j, :], outT[0:96, j, :], sh[0:96, :])
                yield

            _PTS = {}

            def gen_s(j):
                """Attention scores + exp for head pair j (heads 2j, 2j+1)."""
                # qz fill (complement halves are persistent zeros)
                sl = j % 2
                nc.vector.tensor_copy(qza[0:64, sl, :], qT[0:64, j, :])
                nc.vector.tensor_copy(qzb[64:128, sl, :], qT[64:128, j, :])
                pts = {}
                for nk in range(6):
                    qlo, qhi, moff = R0SUB[nk]
                    for e, qz in ((0, qza), (1, qzb)):
                        st = psS.tile([128, 1024], f32, tag=f"st{e}")
                        lhs = kT[:, j, nk * 128:(nk + 1) * 128]
                        nc.tensor.matmul(st[:, qlo:qhi], lhs, qz[:, sl, qlo:qhi],
                                         start=True, stop=True)
                        nc.tensor.matmul(st[:, 512:770], lhs, qz[:, sl, 512:770],
                                         start=True, stop=True)
                        # pt col c <-> q position qlo + c (variable width)
                        pt = ptp.tile([128, TP - qlo], bf16, tag=f"pt{e}_{nk}")
                        pts[(e, nk)] = pt
                        nc.scalar.activation(out=pt[:, 0:TP - qlo],
                                             in_=st[:, qlo:770],
                                             func=Exp, scale=0.125)
                        if moff is not None:
                            nc.gpsimd.tensor_mul(
                                pt[:, 0:128], pt[:, 0:128], m0_sb)
                    yield
                # kv row 768: q cols 513:769 allowed (col 0 of pt6 = q512 = 0)
                for e, qz in ((0, qza), (1, qzb)):
                    st = psS.tile([128, 1024], f32, tag=f"st{e}")
                    nc.tensor.matmul(st[0:1, 0:258], kT[:, j, 768:769],
                                     qz[:, sl, 512:770], start=True, stop=True)
                    pt6 = ptp.tile([128, 258], bf16, tag=f"pt6_{e}")
                    pts[(e, 6)] = pt6
                    nc.vector.memset(pt6[0:1, 0:1], 0.0)
                    nc.scalar.activation(out=pt6[0:1, 1:258],
                                         in_=st[0:1, 1:258],
                                         func=Exp, scale=0.125)
                yield
                _PTS[j] = pts

            def gen_v(tt):
                tsz = 128 if tt < 6 else 1
                ps = psP.tile([128, 1024], f32, tag="ps")
                for cig in range(4):
                    for ci in (2 * cig, 2 * cig + 1):
                        lhs = xkv[:, ci, tt * 128:tt * 128 + tsz]
                        for hf in (0, 1):
                            nc.tensor.matmul(
                                ps[:tsz, hf * 512:hf * 512 + 512],
                                lhs, wv[ci][hf],
                                start=(ci == 0), stop=(ci == 7))
                    yield
                va = vaug[:tsz, tt, :].rearrange("p (pr w) -> p pr w", w=PW)
                ps4 = ps[:tsz, :].rearrange("p (pr e d) -> p pr e d", e=2, d=HD)
                if use_bias:
                    bv4 = bv_sb[:tsz, :].rearrange(
                        "p (pr e d) -> p pr e d", e=2, d=HD)
                    nc.vector.tensor_add(
                        va[:, :, 0:64], ps4[:, :, 0, :], bv4[:, :, 0, :])
                    nc.vector.tensor_add(
                        va[:, :, 66:130], ps4[:, :, 1, :], bv4[:, :, 1, :])
                else:
                    nc.vector.tensor_copy(va[:, :, 0:64], ps4[:, :, 0, :])
                    nc.vector.tensor_copy(va[:, :, 66:130], ps4[:, :, 1, :])
                nc.vector.memset(va[:, :, 64:66], 1.0)
                yield

            def gen_pv(h):
                j, e = h // 2, h % 2
                vs = j * PW + (0 if e == 0 else 2)
                pts = _PTS[j]
                o = psP.tile([128, 1024], f32, tag="ps")
                for nk in range(3):
                    qlo, qhi, _ = R0SUB[nk]
                    nc.tensor.matmul(o[:, qlo:qhi], vaug[:, nk, vs:vs + 128],
                                     pts[(e, nk)][:, 0:qhi - qlo],
                                     start=(nk == 0), stop=False)
                    nc.tensor.matmul(o[:, 512:770], vaug[:, nk, vs:vs + 128],
                                     pts[(e, nk)][:, 512 - qlo:770 - qlo],
                                     start=(nk == 0), stop=False)
                yield
                for nk in range(3, 6):
                    qlo, qhi, _ = R0SUB[nk]
                    nc.tensor.matmul(o[:, qlo:qhi], vaug[:, nk, vs:vs + 128],
                                     pts[(e, nk)][:, 0:qhi - qlo],
                                     start=False, stop=False)
                    nc.tensor.matmul(o[:, 512:770], vaug[:, nk, vs:vs + 128],
                                     pts[(e, nk)][:, 512 - qlo:770 - qlo],
                                     start=False, stop=False)
                nc.tensor.matmul(o[:, 512:770], vaug[0:1, 6, vs:vs + 128],
                                 pts[(e, 6)][0:1, 0:258],
                                 start=False, stop=True)
                # evict unnormalized y (partition-aligned by construction);
                # den row staged through SBUF (DMA can't source PSUM)
                sl2 = j % 2
                if e == 0:
                    nc.vector.tensor_copy(yT[0:64, j, :], o[0:64, 0:770])
                    nc.vector.tensor_copy(stg[64:65, sl2, :], o[64:65, 0:770])
                    nc.sync.dma_start(out=dnd[h:h + 1, :],
                                      in_=stg[64:65, sl2, :])
                else:
                    nc.vector.tensor_copy(yT[64:128, j, :], o[64:128, 0:770])
                    # b_den sits at partition 63; engine APs need 32-aligned
                    # bases, so copy the aligned 32-row block and DMA row 63
                    nc.vector.tensor_copy(stg[32:64, sl2, :], o[32:64, 0:770])
                    nc.sync.dma_start(out=dnd[h:h + 1, :],
                                      in_=stg[63:64, sl2, :])
                yield

            def div_fetch(j):
                rdbc = rdp.tile([128, TP], f32, tag="rdbc")
                nc.gpsimd.dma_start(
                    out=rdbc[0:64, :],
                    in_=dnd[2 * j:2 * j + 1, :].broadcast_to((64, TP)))
                nc.gpsimd.dma_start(
                    out=rdbc[64:128, :],
                    in_=dnd[2 * j + 1:2 * j + 2, :].broadcast_to((64, TP)))
                return rdbc

            def div_apply(j, rdbc):
                nc.vector.reciprocal_approx_fast(out=rdbc, in_=rdbc)
                nc.vector.tensor_mul(yT[0:64, j, :], yT[0:64, j, :],
                                     rdbc[0:64, :])
                nc.vector.tensor_mul(yT[64:128, j, :], yT[64:128, j, :],
                                     rdbc[64:128, :])

            def gen_o(co):
                ps = psP.tile([128, 1024], f32, tag="ps")
                for cig in range(4):
                    for ci in (2 * cig, 2 * cig + 1):
                        lhs = wp[ci][co // 4][:, (co % 4) * 128:(co % 4 + 1) * 128]
                        nc.tensor.matmul(ps[:, 0:512], lhs, yT[:, ci, 0:512],
                                         start=(ci == 0), stop=(ci == 7))
                        nc.tensor.matmul(ps[:, 512:770], lhs, yT[:, ci, 512:770],
                                         start=(ci == 0), stop=(ci == 7))
                    yield
                ot = ooutp.tile([128, TP], f32, tag="ot")
                if use_bias:
                    nc.vector.tensor_scalar_add(
                        ot[:, :], ps[:, 0:770], bp_sb[:, co:co + 1])
                else:
                    nc.vector.tensor_copy(ot[:, :], ps[:, 0:770])
                nc.sync.dma_start(out=out_d[co * 128:(co + 1) * 128, :], in_=ot)
                yield

            # ---------- the weave ----------
            # Emission order IS per-engine queue order; ordering constraints:
            #   QK(j) before S(j); all V before any PV; PV pair j before
            #   S(j+3) (pt pool bufs=3); divisions trail their PV by ~one
            #   pair so the DRAM denominator roundtrip is hidden.
            def drain(g):
                for _ in g:
                    pass

            def pull(n):
                while n > 0 and fillers:
                    try:
                        next(fillers[0])
                        n -= 1
                    except StopIteration:
                        fillers.pop(0)

            drain(gen_qk("q", 0))
            drain(gen_qk("k", 0))
            drain(gen_qk("q", 1))
            drain(gen_qk("k", 1))

            fillers = [gen_v(tt) for tt in range(NTT)]
            rdbcs = {}

            for j in range(8):
                for _ in gen_s(j):
                    pull(2)
                fillers.append(gen_pv(2 * j))
                fillers.append(gen_pv(2 * j + 1))
                if j >= 3:
                    rdbcs[j - 3] = div_fetch(j - 3)
                if j >= 4:
                    div_apply(j - 4, rdbcs.pop(j - 4))
                if j + 2 <= 7:
                    drain(gen_qk("q", j + 2))
                    drain(gen_qk("k", j + 2))
                if j == 5:
                    # output-projection weights into wq's (now free) buffers
                    wp.extend(load_w(wqp, wp_d, "wq", nc.sync))

            # tail: drain remaining PVs, finish divisions
            for j in (5, 6, 7):
                pull(4)
                rdbcs[j] = div_fetch(j)
                pull(4)
                div_apply(j - 1, rdbcs.pop(j - 1))
            while fillers:
                pull(1)
            div_apply(7, rdbcs.pop(7))

            for co in range(8):
                drain(gen_o(co))

    nc.compile()
    return nc


def _host_prep(x_q, x_kv, rotary_pos_emb, Wq, bq, Wk, bk, Wv, bv, Wp, bp,
               use_bias):
    import ml_dtypes
    bf = ml_dtypes.bfloat16
    f = np.float32
    x_q = np.asarray(x_q, f)
    x_kv = np.asarray(x_kv, f)
    freqs = np.asarray(rotary_pos_emb, f)

    # Even/odd pair-split permutation of the first 32 dims of each head, so
    # rotate_half becomes a 16-partition block swap on chip.
    perm = np.arange(C)
    for h in range(H):
        b0 = h * HD
        blk = np.empty(HD, np.int64)
        blk[0:16] = b0 + np.arange(0, 32, 2)
        blk[16:32] = b0 + np.arange(1, 32, 2)
        blk[32:64] = b0 + np.arange(32, 64)
        perm[b0:b0 + HD] = blk

    def wT(W, p=None):
        W = np.asarray(W, f)
        if p is not None:
            W = W[p, :]
        return np.ascontiguousarray(W.T).astype(bf)

    cosE = np.cos(freqs[:, 0::2]).T
    cosO = np.cos(freqs[:, 1::2]).T
    sinE = -np.sin(freqs[:, 0::2]).T
    sinO = np.sin(freqs[:, 1::2]).T
    cosP = np.ones((128, TP), f)
    sinP = np.zeros((128, TP), f)
    for s in (0, 64):
        cosP[s:s + 16, :T] = cosE
        cosP[s + 16:s + 32, :T] = cosO
        sinP[s:s + 16, :T] = sinE
        sinP[s + 16:s + 32, :T] = sinO

    p_idx = np.arange(128)[:, None]
    f_idx = np.arange(128)[None, :]
    m0 = (p_idx < f_idx).astype(f)

    shared = {
        "wqT": wT(Wq, perm),
        "wkT": wT(Wk, perm),
        "wvT": wT(Wv),
        "wpT": wT(Wp),
        "cosP": cosP.astype(bf),
        "sinP": sinP.astype(bf),
        "m0": m0.astype(bf),
    }
    if use_bias:
        bqp = np.asarray(bq, f)[perm]
        bkp = np.asarray(bk, f)[perm]
        shared["bq2"] = np.ascontiguousarray(bqp.reshape(NCI, 128).T)
        shared["bk2"] = np.ascontiguousarray(bkp.reshape(NCI, 128).T)
        shared["bp2"] = np.ascontiguousarray(
            np.asarray(bp, f).reshape(NCI, 128).T)
        shared["bv1"] = np.asarray(bv, f).reshape(1, C).copy()

    def padT(xt):
        out = np.zeros((C, TP), f)
        out[:, :T] = xt
        return out.astype(bf)

    in_maps = []
    for b in range(B):
        m = dict(shared)
        m["xqT"] = padT(x_q[b].T)
        m["xkvT"] = padT(x_kv[b].T)
        in_maps.append(m)
    return in_maps


def kernel(x_q, x_kv, rotary_pos_emb, Wq, bq, Wk, bk, Wv, bv, Wp, bp):
    from concourse.bass_utils import run_bass_kernel_spmd

    use_bias = any(np.any(np.asarray(b)) for b in (bq, bk, bv, bp))
    key = ("nc", use_bias)
    if key not in _CACHE:
        _CACHE[key] = _build_program(use_bias)
    nc = _CACHE[key]

    in_maps = _host_prep(x_q, x_kv, rotary_pos_emb,
                         Wq, bq, Wk, bk, Wv, bv, Wp, bp, use_bias)
    trace = os.environ.get("BTK_TRACE", "0") == "1"
    res = run_bass_kernel_spmd(
        nc, in_maps, core_ids=list(range(B)), trace=trace)
    _CACHE["last_result"] = res
    return np.stack(
        [np.ascontiguousarray(r["out"][:, :T].T.astype(np.float32))
         for r in res.results], axis=0)
